# revision 5
# baseline (speedup 1.0000x reference)
"""CrystalEncoder Trainium2 kernel (v2).

Strategy: pure data parallel — one crystal (batch element) per NeuronCore.
The O(N) geometry (cart coords, pairwise d^2 / d feature rows, flattening)
is done on HOST; the O(N^2 * H) work (RBF expansion, gated message passing)
runs on-device in one fused Bass/Tile kernel.

Device dataflow per core (N=256 atoms, H=128, BINS=40, NL=2):
  1. rf64 [64, 2048] f32 input: 16 "fills" x 4 rows (d^2/d x 2 i-groups),
     each fill covering 8 i-rows x 256 j pairs per group. One cheap DMA
     (cost scales with per-partition bytes, so 64-partition packing wins).
  2. RBF exponents via K=64 matmuls: cE64 [64, 16*128] holds 16 per-fill
     selector blocks; exponent e = -g*d^2 + 2*g*c_k*d per (bin, group),
     bias -g*c_k^2 folded into the Exp activation; rbfT [128, 32768] bf16
     resident (groups at partitions 0/64).
  3. Per layer l: gate matmul (K=40 bf16, ewR stationary per group);
     softplus as Exp then Ln(x+1) in the single natural_log_exp table set
     (Ln over 8192-wide chunks); DVE/Pool multiply by broadcast h_j and
     2-step reduce over j -> aggT; node update zT = node_w^T @ aggT,
     Silu, residual + mask.
  4. Pooling: reduce over atoms -> sum_h [H, 1] -> DRAM.
Host: g = sum_h / (n_valid + 1e-6); mu / log_var projections.

Sync discipline: this walrus build supports at most ONE semaphore wait per
instruction; _install_wait_splitter() splits multi-wait instructions with
same-engine NoOp carriers (as the baseline did).
"""

import numpy as np
import ml_dtypes

B, N, H, LAT, NL, BINS = 8, 256, 128, 64, 2, 40
VMAX = 8.0
GAMMA = 1.0 / (VMAX / BINS) ** 2  # 25.0

G = 2                  # i-groups; bins at partition offsets 0 / 64
IPG = N // G           # 128 i-rows per group
LOCF = IPG * N         # 32768 pairs per group (free size of rbfT)
NFILL = 16             # rf fills
FILLP = 2048           # pairs per fill per group
IPF = FILLP // N       # 8 i-rows per fill per group
ECH = 2048             # pairs per Exp chunk in rbf stage (= FILLP)
BLK = 8192             # pairs per gate block (per group); 4 blocks per group
IPB = BLK // N         # 32 i-rows per block
MMF = 512              # matmul free size (one PSUM bank of f32)

# mul/add placement: 'pool' = tensor ops on gpsimd engine, 'dve' = on vector
MUL_ENGINE = "pool"

_CACHE = {}


def _install_wait_splitter():
    """This walrus build supports at most ONE semaphore wait per ISA
    instruction. Split every multi-wait instruction by inserting same-engine
    NoOp carriers, each holding one of the waits, immediately before it."""
    import bass_rust
    import concourse.tile as tile
    from concourse import mybir

    if getattr(tile.TileContext, "_wait_split_installed", False):
        return
    orig = tile.TileContext._lower_ordered_insts
    counter = [0]

    def patched(self, ordered):
        for insts in ordered.values():
            newl = []
            for inst in insts:
                si = inst.sync_info
                ow = list(si.on_wait) if (si is not None and si.on_wait) else []
                if len(ow) > 1 and inst.engine != mybir.EngineType.Unassigned:
                    for w in ow[:-1]:
                        counter[0] += 1
                        nop = bass_rust.InstNoOp(
                            name=f"wsplit_{counter[0]}", ins=[], outs=[]
                        )
                        nop.engine = inst.engine
                        nop.sync_info = bass_rust.SyncInfo(
                            on_wait=[w], on_update=[]
                        )
                        newl.append(nop)
                    inst.sync_info = bass_rust.SyncInfo(
                        on_wait=[ow[-1]], on_update=list(si.on_update or [])
                    )
                newl.append(inst)
            insts[:] = newl
        return orig(self, ordered)

    tile.TileContext._lower_ordered_insts = patched

    def patched_dab(self, tick_clock, wait_clock):
        from concourse.vector_clock import ScopedClock

        probe = self.nc.sync.nop()
        wait_clock.add_sem_waits(
            probe.ins, ScopedClock({None: tick_clock.global_clock})
        )
        si = probe.ins.sync_info
        ow = list(si.on_wait) if (si is not None and si.on_wait) else []
        if len(ow) > 1:
            probe.ins.sync_info = bass_rust.SyncInfo(
                on_wait=[ow[0]], on_update=list(si.on_update or [])
            )
            for w in ow[1:]:
                n2 = self.nc.sync.nop()
                n2.ins.sync_info = bass_rust.SyncInfo(on_wait=[w], on_update=[])
        self.nc.sync.drain()
        self.nc.all_engine_barrier()
        popped = self.nc._tile_sem_poison_stack.pop()
        assert popped is self._sem_poison
        self.nc.clear_and_free_semaphores(list(self.sems.allocated().values()))
        self.nc.all_engine_barrier()

    tile.TileContext._drain_and_barrier = patched_dab
    tile.TileContext._wait_split_installed = True


def _build_nc(reps=1):
    import concourse.bass as bass
    import concourse.tile as tile
    from concourse import mybir

    _install_wait_splitter()

    F32 = mybir.dt.float32
    BF16 = mybir.dt.bfloat16
    AF = mybir.ActivationFunctionType
    X = mybir.AxisListType
    ALU = mybir.AluOpType
    POOL = mybir.EngineType.Pool

    nc = bass.Bass("TRN2", target_bir_lowering=False, debug=False)

    def dep_nop(engine, aps):
        """Engine-local nop reading `aps`: pulls their producers' ticks into
        the engine's observed clock so later real instructions need at most
        one new semaphore wait."""
        nop = engine.nop(hint="dep").ins
        nop.ins = [engine.lower_ap(ap) for ap in aps]
        return nop

    F32R = mybir.dt.float32r
    d_rf = nc.dram_tensor("rf64", [64, NFILL * FILLP // 16], F32R,
                          kind="ExternalInput")  # [64, 2048]
    d_cE = nc.dram_tensor("cE64", [64, NFILL * H], F32R, kind="ExternalInput")
    d_cbias = nc.dram_tensor("cbias", [H, 1], F32, kind="ExternalInput")
    d_ewR = nc.dram_tensor("ewR", [H, NL * H], BF16, kind="ExternalInput")
    d_ebT = nc.dram_tensor("ebT", [H, NL], F32, kind="ExternalInput")
    d_nwT = nc.dram_tensor("nwT", [H, NL * H], F32, kind="ExternalInput")
    d_nbT = nc.dram_tensor("nbT", [H, NL], F32, kind="ExternalInput")
    d_h0T = nc.dram_tensor("h0T", [H, N], F32, kind="ExternalInput")
    d_maskF = nc.dram_tensor("maskF", [H, N], F32, kind="ExternalInput")
    d_sumh = nc.dram_tensor("sumh", [H, 1], F32, kind="ExternalOutput")

    mul_eng = nc.gpsimd if MUL_ENGINE == "pool" else nc.vector

    with tile.TileContext(nc) as tc:
        with tc.tile_pool(name="consts", bufs=1) as consts:
            SP = mybir.EngineType.SP
            kwp = dict(forced_dma_engine=POOL)
            kws = dict(forced_dma_engine=SP)
            # rf64 / cE64 gate the first matmuls: load them first, on
            # separate queues; the rest follows split across both queues.
            t_rf = consts.tile_from(d_rf[:], **kwp)
            t_cE = consts.tile_from(d_cE[:], **kws)
            t_cbias = consts.tile_from(d_cbias[:], **kwp)
            t_ewR = consts.tile_from(d_ewR[:], **kws)
            t_ebT = consts.tile_from(d_ebT[:], **kwp)
            t_nwT = consts.tile_from(d_nwT[:], **kws)
            t_nbT = consts.tile_from(d_nbT[:], **kwp)
            t_hT = consts.tile_from(d_h0T[:], **kws)
            t_maskF = consts.tile_from(d_maskF[:], **kwp)

            rbfT = consts.tile([H, LOCF], BF16)

            # every engine pre-observes the (single) DMA proc at its max tick
            dep_nop(nc.tensor, [t_rf[:], t_cE[:], t_ewR[:], t_nwT[:]])
            dep_nop(nc.scalar, [t_cbias[:], t_ebT[:], t_nbT[:]])
            dep_nop(nc.vector, [t_hT[:], t_maskF[:]])
            dep_nop(nc.gpsimd, [t_hT[:], t_maskF[:]])

            h00 = consts.tile([H, N], F32, tag="h00")
            nc.vector.tensor_copy(h00[:], t_hT[:])

            for rep in range(reps):
              if rep > 0:
                nc.vector.tensor_copy(t_hT[:], h00[:])

              # ---- stage 2: resident RBF table from host rf rows ----
              with tc.tile_pool(name="eps", bufs=2, space="PSUM") as eps:
                  for f in range(NFILL):
                      e = eps.tile([H, ECH], F32, tag="eps")
                      for s in range(ECH // MMF):
                          nc.tensor.matmul(
                              e[:, s * MMF:(s + 1) * MMF],
                              t_cE[:, f * H:(f + 1) * H],
                              t_rf[:, s * MMF:(s + 1) * MMF],
                              start=True, stop=True,
                          )
                      nc.scalar.activation(
                          rbfT[:, f * ECH:(f + 1) * ECH], e[:], AF.Exp,
                          bias=t_cbias[:],
                      )

              # ---- stage 3: message-passing layers ----
              with tc.tile_pool(name="lay", bufs=1) as lay, \
                   tc.tile_pool(name="gxp", bufs=2) as gxp, \
                   tc.tile_pool(name="gtp", bufs=2) as gtp, \
                   tc.tile_pool(name="ppp", bufs=2) as ppp, \
                   tc.tile_pool(name="tmp", bufs=2) as tmpp, \
                   tc.tile_pool(name="gpp", bufs=2, space="PSUM") as gpp:
                  hmr = lay.tile([H, N], BF16, tag="hmr0")
                  nc.vector.tensor_copy(hmr[:], t_hT[:])
                  NBLK = G * (LOCF // BLK)  # 8 blocks of BLK pairs per layer
                  for l in range(NL):
                      aggT = lay.tile([H, N], F32, tag=f"agg{l}")
                      for b in range(NBLK):
                          # block order: all g0 blocks, then all g1 blocks
                          g, bi = divmod(b, LOCF // BLK)
                          lf = bi * BLK
                          last = b == NBLK - 1
                          gx = gxp.tile([H, BLK], BF16, tag="gx")
                          for c in range(BLK // ECH):
                              gp = gpp.tile([H, ECH], F32, tag="gp")
                              for s in range(ECH // MMF):
                                  f0 = lf + c * ECH + s * MMF
                                  nc.tensor.matmul(
                                      gp[:, s * MMF:(s + 1) * MMF],
                                      t_ewR[64 * g:64 * g + BINS,
                                            l * H:(l + 1) * H],
                                      rbfT[64 * g:64 * g + BINS,
                                           f0:f0 + MMF],
                                      start=True, stop=True,
                                  )
                              nc.scalar.activation(
                                  gx[:, c * ECH:(c + 1) * ECH], gp[:],
                                  AF.Exp, bias=t_ebT[:, l:l + 1],
                              )
                          # softplus(x) = ln(exp(x) + 1); same ACT table set
                          gt = gtp.tile([H, BLK], BF16, tag="gt")
                          i0 = g * IPG + bi * IPB
                          # DVE runs at 2x on bf16; Pool has no fast mode but
                          # is otherwise idle. Split blocks between them, and
                          # pipeline the last block in ECH sub-chunks on DVE
                          # so the end-of-layer serial tail stays short.
                          eng = nc.vector if b in (5, 6) or last else nc.gpsimd
                          if not last:
                              nc.scalar.activation(gt[:], gx[:], AF.Ln,
                                                   bias=1.0)
                              pp = ppp.tile([H, BLK], BF16, tag="pp")
                              eng.tensor_mul(
                                  pp[:].rearrange("p (r c) -> p r c", c=N),
                                  gt[:].rearrange("p (r c) -> p r c", c=N),
                                  hmr[:, None, :].broadcast_to([H, IPB, N]),
                              )
                              tm = tmpp.tile([H, BLK // 2], BF16, tag="tm")
                              tmv = tm[:].rearrange(
                                  "p (r c) -> p r c", c=N // 2)
                              ppv = pp[:].rearrange("p (r c) -> p r c", c=N)
                              eng.tensor_add(
                                  tmv, ppv[:, :, 0:N // 2],
                                  ppv[:, :, N // 2:N],
                              )
                              nc.vector.reduce_sum(
                                  out=aggT[:, i0:i0 + IPB], in_=tmv,
                                  axis=X.X,
                              )
                          else:
                              pp = ppp.tile([H, BLK], BF16, tag="pp")
                              tm = tmpp.tile([H, BLK // 2], BF16, tag="tm")
                              for c in range(BLK // ECH):
                                  sl_ = slice(c * ECH, (c + 1) * ECH)
                                  nc.scalar.activation(
                                      gt[:, sl_], gx[:, sl_], AF.Ln,
                                      bias=1.0,
                                  )
                                  ipc = ECH // N
                                  ppv = pp[:, sl_].rearrange(
                                      "p (r c) -> p r c", c=N)
                                  eng.tensor_mul(
                                      ppv,
                                      gt[:, sl_].rearrange(
                                          "p (r c) -> p r c", c=N),
                                      hmr[:, None, :].broadcast_to(
                                          [H, ipc, N]),
                                  )
                                  tmv = tm[:, c * ECH // 2:(c + 1) * ECH // 2
                                           ].rearrange(
                                      "p (r c) -> p r c", c=N // 2)
                                  eng.tensor_add(
                                      tmv, ppv[:, :, 0:N // 2],
                                      ppv[:, :, N // 2:N],
                                  )
                                  j0 = i0 + c * ipc
                                  nc.vector.reduce_sum(
                                      out=aggT[:, j0:j0 + ipc], in_=tmv,
                                      axis=X.X,
                                  )
                      # node update
                      dep_nop(nc.tensor, [aggT[:]])
                      zp = gpp.tile([H, ECH], F32, tag="gp")
                      nc.tensor.matmul(
                          zp[:, :N], t_nwT[:, l * H:(l + 1) * H], aggT[:],
                          start=True, stop=True,
                      )
                      sl = lay.tile([H, N], F32, tag=f"sil{l}")
                      nc.scalar.activation(
                          sl[:], zp[:, :N], AF.Silu, bias=t_nbT[:, l:l + 1],
                      )
                      h2 = lay.tile([H, N], F32, tag=f"h2_{l}")
                      nc.vector.tensor_add(h2[:], t_hT[:], sl[:])
                      nc.vector.tensor_mul(t_hT[:], h2[:], t_maskF[:])
                      if l + 1 < NL:
                          hmr = lay.tile([H, N], BF16, tag=f"hmr{l + 1}")
                          nc.vector.tensor_copy(hmr[:], t_hT[:])

                  sumh = lay.tile([H, 1], F32, tag="sumh")
                  nc.vector.reduce_sum(out=sumh[:], in_=t_hT[:], axis=X.X)
                  nc.gpsimd.dma_start(out=d_sumh[:], in_=sumh[:])

    return nc


def _get_nc(reps=1):
    key = f"nc{reps}"
    if key not in _CACHE:
        _CACHE[key] = _build_nc(reps)
    return _CACHE[key]


def _shared_inputs(edge_w, edge_b, node_w, node_b):
    centers = np.linspace(0.0, VMAX, BINS).astype(np.float64)
    # cE64: 16 per-fill selector blocks. Fill f uses rf rows 4f..4f+3:
    # row 4f+2g+0 = d^2 of group g, row 4f+2g+1 = d of group g.
    cE = np.zeros((64, NFILL * H), np.float32)
    for f in range(NFILL):
        for g in range(G):
            col0 = f * H + 64 * g
            cE[4 * f + 2 * g + 0, col0:col0 + BINS] = -GAMMA
            cE[4 * f + 2 * g + 1, col0:col0 + BINS] = 2.0 * GAMMA * centers
    cbias = np.zeros((H, 1), np.float32)
    ewR = np.zeros((H, NL * H), np.float32)
    for g in range(G):
        cbias[64 * g:64 * g + BINS, 0] = -GAMMA * centers * centers
        for l in range(NL):
            ewR[64 * g:64 * g + BINS, l * H:(l + 1) * H] = edge_w[l]
    ewR = ewR.astype(ml_dtypes.bfloat16)
    ebT = np.ascontiguousarray(edge_b.T).astype(np.float32)      # [H, NL]
    nwT = np.concatenate([node_w[l] for l in range(NL)], axis=1)
    nwT = np.ascontiguousarray(nwT).astype(np.float32)           # [H, NL*H]
    nbT = np.ascontiguousarray(node_b.T).astype(np.float32)      # [H, NL]
    return dict(cE64=cE, cbias=cbias, ewR=ewR, ebT=ebT, nwT=nwT, nbT=nbT)


def make_in_maps(atom_types, frac_coords, lattice, mask, emb_table,
                 edge_w, edge_b, node_w, node_b):
    shared = _shared_inputs(edge_w, edge_b, node_w, node_b)
    in_maps = []
    for b in range(B):
        cart = (frac_coords[b] @ lattice[b]).astype(np.float32)  # (N, 3)
        nsq = (cart * cart).sum(-1).astype(np.float32)
        d2 = nsq[:, None] + nsq[None, :] - 2.0 * (cart @ cart.T)
        d2 = np.maximum(d2, 0.0).astype(np.float32) + np.float32(1e-6)
        d = np.sqrt(d2)
        # rf64 [64, 2048]: fill f rows 4f+2g+{0,1} = (d^2, d) of group g,
        # i-rows [8f, 8f+8) of group g, row-major over (i, j).
        rf = np.empty((64, FILLP), np.float32)
        for f in range(NFILL):
            for g in range(G):
                i0 = g * IPG + f * IPF
                rf[4 * f + 2 * g + 0] = d2[i0:i0 + IPF].reshape(-1)
                rf[4 * f + 2 * g + 1] = d[i0:i0 + IPF].reshape(-1)
        types = np.where(mask[b], atom_types[b], 0).astype(np.int64)
        h0T = np.ascontiguousarray(emb_table[types].T).astype(np.float32)
        maskF = np.broadcast_to(
            mask[b].astype(np.float32)[None, :], (H, N)
        ).copy()
        in_maps.append(dict(rf64=rf, h0T=h0T, maskF=maskF, **shared))
    return in_maps


def kernel(**inputs):
    from concourse.bass_utils import run_bass_kernel_spmd

    atom_types = np.asarray(inputs["atom_types"])
    frac_coords = np.asarray(inputs["frac_coords"], np.float32)
    lattice = np.asarray(inputs["lattice"], np.float32)
    mask = np.asarray(inputs["mask"]).astype(bool)
    emb_table = np.asarray(inputs["emb_table"], np.float32)
    edge_w = np.asarray(inputs["edge_w"], np.float32)
    edge_b = np.asarray(inputs["edge_b"], np.float32)
    node_w = np.asarray(inputs["node_w"], np.float32)
    node_b = np.asarray(inputs["node_b"], np.float32)
    mu_w = np.asarray(inputs["mu_w"], np.float32)
    mu_b = np.asarray(inputs["mu_b"], np.float32)
    var_w = np.asarray(inputs["var_w"], np.float32)
    var_b = np.asarray(inputs["var_b"], np.float32)

    nc = _get_nc()
    in_maps = make_in_maps(atom_types, frac_coords, lattice, mask, emb_table,
                           edge_w, edge_b, node_w, node_b)
    res = run_bass_kernel_spmd(nc, in_maps, core_ids=list(range(B)))
    sum_h = np.stack([res.results[b]["sumh"][:, 0] for b in range(B)])
    n_valid = mask.sum(1).astype(np.float32)
    g = sum_h / (n_valid[:, None] + 1e-6)
    mu = (g @ mu_w + mu_b).astype(np.float32)
    log_var = (g @ var_w + var_b).astype(np.float32)
    return mu, log_var


# revision 15
# speedup vs baseline: 182.0957x; 182.0957x over previous
"""CrystalEncoder Trainium2 kernel (v3): all 8 crystals on ONE NeuronCore.

Why one core: in this axon environment each per-device NEFF dispatch carries
~1.2ms of launch overhead and the 8-device dispatch serializes them (~10ms
total — which is what the 9.25ms baseline number actually was). One dispatch
running all 8 crystals sequentially costs 1 launch + 8x ~0.3ms of compute.

Host does the O(N) / O(N^2) scalar geometry (cart coords, pairwise d^2 / d
feature rows); device does everything O(N^2*H).

Per crystal (N=256 atoms, H=128, BINS=40, NL=2):
  1. rf64 [64, 2048] f32r slice per crystal: 16 "fills" x 4 rows
     (d^2/d x 2 i-groups), each fill = 8 i-rows x 256 j pairs per group.
  2. RBF exponents via K=64 matmuls (cE64 holds 16 per-fill selector
     blocks); Exp with bias -g*c_k^2 -> rbfT [128, 32768] bf16 (groups at
     partitions 0/64).
  3. Per layer: gate matmuls (K=40 bf16, 512-free, psum 2048-chunks);
     softplus = Exp then Ln(1+x), both in the natural_log_exp table set;
     DVE (2x bf16) multiply by broadcast h_j, add-halves, reduce -> aggT;
     node update zT = node_w^T @ aggT + Silu + residual + mask.
  4. sum over atoms -> sumh column; one [H, 8] output DMA at the end.

Software pipelining: crystal c's layer-2 node update is deferred until
after crystal c+1's RBF stage is emitted, and layer-2's first two gate
blocks are produced before layer-1's node update — so ACT (the bottleneck
engine, ~64% of cycles) never waits on the DVE reduce tails.

All element-wise tensor work is on DVE: GpSimd tensor ops are Q7 software
at ~0.42 efficiency on real HW (measured 2.4x the cost-model estimate).

Sync discipline: this walrus build supports at most ONE semaphore wait per
instruction; _install_wait_splitter() splits multi-wait instructions with
same-engine NoOp carriers.
"""

import numpy as np
import ml_dtypes

B, N, H, LAT, NL, BINS = 8, 256, 128, 64, 2, 40
VMAX = 8.0
GAMMA = 1.0 / (VMAX / BINS) ** 2  # 25.0

G = 2                  # i-groups; bins at partition offsets 0 / 64
IPG = N // G           # 128 i-rows per group
LOCF = IPG * N         # 32768 pairs per group (free size of rbfT)
NFILL = 16             # rf fills per crystal
FILLP = 2048           # pairs per fill per group
IPF = FILLP // N       # 8 i-rows per fill per group
ECH = 2048             # pairs per Exp chunk (= one PSUM tile)
BLK = 8192             # pairs per gate block (per group); 4 blocks/group
IPB = BLK // N         # 32 i-rows per block
MMF = 512              # matmul free size (hard ISA limit)
NBLK = G * (LOCF // BLK)  # 8 gate blocks per layer

_CACHE = {}


def _install_wait_splitter():
    """This walrus build supports at most ONE semaphore wait per ISA
    instruction. Split every multi-wait instruction by inserting same-engine
    NoOp carriers, each holding one of the waits, immediately before it."""
    import bass_rust
    import concourse.tile as tile
    from concourse import mybir

    if getattr(tile.TileContext, "_wait_split_installed", False):
        return
    orig = tile.TileContext._lower_ordered_insts
    counter = [0]

    def patched(self, ordered):
        for insts in ordered.values():
            newl = []
            for inst in insts:
                si = inst.sync_info
                ow = list(si.on_wait) if (si is not None and si.on_wait) else []
                if len(ow) > 1 and inst.engine != mybir.EngineType.Unassigned:
                    for w in ow[:-1]:
                        counter[0] += 1
                        nop = bass_rust.InstNoOp(
                            name=f"wsplit_{counter[0]}", ins=[], outs=[]
                        )
                        nop.engine = inst.engine
                        nop.sync_info = bass_rust.SyncInfo(
                            on_wait=[w], on_update=[]
                        )
                        newl.append(nop)
                    inst.sync_info = bass_rust.SyncInfo(
                        on_wait=[ow[-1]], on_update=list(si.on_update or [])
                    )
                newl.append(inst)
            insts[:] = newl
        return orig(self, ordered)

    tile.TileContext._lower_ordered_insts = patched

    def patched_dab(self, tick_clock, wait_clock):
        from concourse.vector_clock import ScopedClock

        probe = self.nc.sync.nop()
        wait_clock.add_sem_waits(
            probe.ins, ScopedClock({None: tick_clock.global_clock})
        )
        si = probe.ins.sync_info
        ow = list(si.on_wait) if (si is not None and si.on_wait) else []
        if len(ow) > 1:
            probe.ins.sync_info = bass_rust.SyncInfo(
                on_wait=[ow[0]], on_update=list(si.on_update or [])
            )
            for w in ow[1:]:
                n2 = self.nc.sync.nop()
                n2.ins.sync_info = bass_rust.SyncInfo(on_wait=[w], on_update=[])
        self.nc.sync.drain()
        self.nc.all_engine_barrier()
        popped = self.nc._tile_sem_poison_stack.pop()
        assert popped is self._sem_poison
        self.nc.clear_and_free_semaphores(list(self.sems.allocated().values()))
        self.nc.all_engine_barrier()

    tile.TileContext._drain_and_barrier = patched_dab
    tile.TileContext._wait_split_installed = True


def _build_nc(n_crystals=B):
    import concourse.bass as bass
    import concourse.tile as tile
    from concourse import mybir

    _install_wait_splitter()

    F32 = mybir.dt.float32
    F32R = mybir.dt.float32r
    BF16 = mybir.dt.bfloat16
    AF = mybir.ActivationFunctionType
    X = mybir.AxisListType
    POOL = mybir.EngineType.Pool
    SP = mybir.EngineType.SP

    nc = bass.Bass("TRN2", target_bir_lowering=False, debug=False)

    def dep_nop(engine, aps):
        """Engine-local nop reading `aps`: pulls their producers' ticks into
        the engine's observed clock so later real instructions need at most
        one new semaphore wait."""
        nop = engine.nop(hint="dep").ins
        nop.ins = [engine.lower_ap(ap) for ap in aps]
        return nop

    NC = n_crystals
    d_rf = nc.dram_tensor("rf64", [64, NC * FILLP], F32R, kind="ExternalInput")
    d_cE = nc.dram_tensor("cE64", [64, NFILL * H], F32R, kind="ExternalInput")
    d_cbias = nc.dram_tensor("cbias", [H, 1], F32, kind="ExternalInput")
    d_ewR = nc.dram_tensor("ewR", [H, NL * H], BF16, kind="ExternalInput")
    d_ebT = nc.dram_tensor("ebT", [H, NL], F32, kind="ExternalInput")
    d_nwT = nc.dram_tensor("nwT", [H, NL * H], F32, kind="ExternalInput")
    d_nbT = nc.dram_tensor("nbT", [H, NL], F32, kind="ExternalInput")
    d_h0T = nc.dram_tensor("h0T", [H, NC * N], F32, kind="ExternalInput")
    d_maskF = nc.dram_tensor("maskF", [H, NC * N], F32, kind="ExternalInput")
    d_sumh = nc.dram_tensor("sumh", [H, NC], F32, kind="ExternalOutput")

    with tile.TileContext(nc) as tc:
        with tc.tile_pool(name="consts", bufs=1) as consts, \
             tc.tile_pool(name="rfp", bufs=1) as rfp, \
             tc.tile_pool(name="lay", bufs=2) as lay, \
             tc.tile_pool(name="gxp", bufs=2) as gxp, \
             tc.tile_pool(name="gtp", bufs=2) as gtp, \
             tc.tile_pool(name="ppp", bufs=1) as ppp, \
             tc.tile_pool(name="tmp", bufs=1) as tmpp, \
             tc.tile_pool(name="ps", bufs=2, space="PSUM") as ps:
            kwp = dict(forced_dma_engine=POOL)
            kws = dict(forced_dma_engine=SP)
            t_cE = consts.tile_from(d_cE[:], **kws)
            t_cbias = consts.tile_from(d_cbias[:], **kwp)
            t_ewR = consts.tile_from(d_ewR[:], **kws)
            t_ebT = consts.tile_from(d_ebT[:], **kwp)
            t_nwT = consts.tile_from(d_nwT[:], **kws)
            t_nbT = consts.tile_from(d_nbT[:], **kwp)
            t_h = consts.tile_from(d_h0T[:], **kws)
            t_maskF = consts.tile_from(d_maskF[:], **kwp)

            rbfT = consts.tile([H, LOCF], BF16)
            sumh = consts.tile([H, NC], F32, tag="sumh")

            dep_nop(nc.tensor, [t_cE[:], t_ewR[:], t_nwT[:]])
            dep_nop(nc.scalar, [t_cbias[:], t_ebT[:], t_nbT[:]])
            dep_nop(nc.vector, [t_h[:], t_maskF[:]])

            def stage2(c):
                """RBF table build for crystal c (PE + ACT; DMA one fill
                set from DRAM)."""
                rf = rfp.tile([64, FILLP], F32R, tag="rf")
                nc.gpsimd.dma_start(
                    out=rf[:], in_=d_rf[:, c * FILLP:(c + 1) * FILLP])
                dep_nop(nc.tensor, [rf[:]])
                for f in range(NFILL):
                    e = ps.tile([H, ECH], F32, tag="ps")
                    for s in range(ECH // MMF):
                        nc.tensor.matmul(
                            e[:, s * MMF:(s + 1) * MMF],
                            t_cE[:, f * H:(f + 1) * H],
                            rf[:, s * MMF:(s + 1) * MMF],
                            start=True, stop=True,
                        )
                    nc.scalar.activation(
                        rbfT[:, f * ECH:(f + 1) * ECH], e[:], AF.Exp,
                        bias=t_cbias[:],
                    )

            def gate_produce(l, b):
                """Gate matmuls + softplus (Exp, Ln) for one 8192 block.
                Returns (gt, g, bi)."""
                g, bi = divmod(b, LOCF // BLK)
                lf = bi * BLK
                gx = gxp.tile([H, BLK], BF16, tag="gx")
                for cch in range(BLK // ECH):
                    gp = ps.tile([H, ECH], F32, tag="ps")
                    for s in range(ECH // MMF):
                        f0 = lf + cch * ECH + s * MMF
                        nc.tensor.matmul(
                            gp[:, s * MMF:(s + 1) * MMF],
                            t_ewR[64 * g:64 * g + BINS, l * H:(l + 1) * H],
                            rbfT[64 * g:64 * g + BINS, f0:f0 + MMF],
                            start=True, stop=True,
                        )
                    nc.scalar.activation(
                        gx[:, cch * ECH:(cch + 1) * ECH], gp[:], AF.Exp,
                        bias=t_ebT[:, l:l + 1],
                    )
                gt = gtp.tile([H, BLK], BF16, tag="gt")
                nc.scalar.activation(gt[:], gx[:], AF.Ln, bias=1.0)
                return gt

            def gate_consume(gt, b, hmr, aggT, split=False):
                """DVE: pp = gt * h_j; add halves; reduce over j -> aggT
                columns. `split` pipelines in ECH sub-chunks to shorten the
                serial tail at layer end."""
                g, bi = divmod(b, LOCF // BLK)
                i0 = g * IPG + bi * IPB
                subs = (BLK // ECH) if split else 1
                w = BLK // subs
                ipw = w // N
                pp = ppp.tile([H, BLK], BF16, tag="pp")
                tm = tmpp.tile([H, BLK // 2], BF16, tag="tm")
                for s in range(subs):
                    sl_ = slice(s * w, (s + 1) * w)
                    ppv = pp[:, sl_].rearrange("p (r c) -> p r c", c=N)
                    nc.vector.tensor_mul(
                        ppv,
                        gt[:, sl_].rearrange("p (r c) -> p r c", c=N),
                        hmr[:, None, :].broadcast_to([H, ipw, N]),
                    )
                    tmv = tm[:, s * w // 2:(s + 1) * w // 2].rearrange(
                        "p (r c) -> p r c", c=N // 2)
                    nc.vector.tensor_add(
                        tmv, ppv[:, :, 0:N // 2], ppv[:, :, N // 2:N])
                    nc.vector.reduce_sum(
                        out=aggT[:, i0 + s * ipw:i0 + (s + 1) * ipw],
                        in_=tmv, axis=X.X,
                    )

            def node_update(c, l, aggT):
                """zT = node_w^T @ aggT; h += silu(zT + b); h *= mask."""
                hsl = slice(c * N, (c + 1) * N)
                dep_nop(nc.tensor, [aggT[:]])
                zp = ps.tile([H, ECH], F32, tag="ps")
                nc.tensor.matmul(
                    zp[:, :N], t_nwT[:, l * H:(l + 1) * H], aggT[:],
                    start=True, stop=True,
                )
                sl = lay.tile([H, N], F32, tag="sil")
                nc.scalar.activation(
                    sl[:], zp[:, :N], AF.Silu, bias=t_nbT[:, l:l + 1])
                h2 = lay.tile([H, N], F32, tag="h2")
                nc.vector.tensor_add(h2[:], t_h[:, hsl], sl[:])
                nc.vector.tensor_mul(t_h[:, hsl], h2[:], t_maskF[:, hsl])

            deferred = None  # (c, aggT2) awaiting layer-2 node update

            def finish_crystal(dfr):
                c, aggT2 = dfr
                node_update(c, 1, aggT2)
                nc.vector.reduce_sum(
                    out=sumh[:, c:c + 1], in_=t_h[:, c * N:(c + 1) * N],
                    axis=X.X,
                )

            for c in range(NC):
                stage2(c)
                if deferred is not None:
                    finish_crystal(deferred)
                    deferred = None
                hsl = slice(c * N, (c + 1) * N)
                hmr1 = lay.tile([H, N], BF16, tag="hmr1")
                nc.vector.tensor_copy(hmr1[:], t_h[:, hsl])
                # layer 1
                aggT1 = lay.tile([H, N], F32, tag="agg1")
                for b in range(NBLK):
                    gt = gate_produce(0, b)
                    gate_consume(gt, b, hmr1, aggT1, split=(b == NBLK - 1))
                # layer 2: produce first two blocks before layer-1 node
                # update so ACT stays busy over the layer-1 reduce tail
                aggT2 = lay.tile([H, N], F32, tag="agg2")
                gt20 = gate_produce(1, 0)
                gt21 = gate_produce(1, 1)
                node_update(c, 0, aggT1)
                hmr2 = lay.tile([H, N], BF16, tag="hmr2")
                nc.vector.tensor_copy(hmr2[:], t_h[:, hsl])
                gate_consume(gt20, 0, hmr2, aggT2)
                gate_consume(gt21, 1, hmr2, aggT2)
                for b in range(2, NBLK):
                    gt = gate_produce(1, b)
                    gate_consume(gt, b, hmr2, aggT2, split=(b == NBLK - 1))
                deferred = (c, aggT2)

            finish_crystal(deferred)
            nc.gpsimd.dma_start(out=d_sumh[:], in_=sumh[:])

    return nc


def _get_nc():
    if "nc" not in _CACHE:
        _CACHE["nc"] = _build_nc()
    return _CACHE["nc"]


def _shared_inputs(edge_w, edge_b, node_w, node_b):
    centers = np.linspace(0.0, VMAX, BINS).astype(np.float64)
    # cE64: 16 per-fill selector blocks. Fill f uses rf rows 4f..4f+3:
    # row 4f+2g+0 = d^2 of group g, row 4f+2g+1 = d of group g.
    cE = np.zeros((64, NFILL * H), np.float32)
    for f in range(NFILL):
        for g in range(G):
            col0 = f * H + 64 * g
            cE[4 * f + 2 * g + 0, col0:col0 + BINS] = -GAMMA
            cE[4 * f + 2 * g + 1, col0:col0 + BINS] = 2.0 * GAMMA * centers
    cbias = np.zeros((H, 1), np.float32)
    ewR = np.zeros((H, NL * H), np.float32)
    for g in range(G):
        cbias[64 * g:64 * g + BINS, 0] = -GAMMA * centers * centers
        for l in range(NL):
            ewR[64 * g:64 * g + BINS, l * H:(l + 1) * H] = edge_w[l]
    ewR = ewR.astype(ml_dtypes.bfloat16)
    ebT = np.ascontiguousarray(edge_b.T).astype(np.float32)      # [H, NL]
    nwT = np.concatenate([node_w[l] for l in range(NL)], axis=1)
    nwT = np.ascontiguousarray(nwT).astype(np.float32)           # [H, NL*H]
    nbT = np.ascontiguousarray(node_b.T).astype(np.float32)      # [H, NL]
    return dict(cE64=cE, cbias=cbias, ewR=ewR, ebT=ebT, nwT=nwT, nbT=nbT)


def make_in_maps(atom_types, frac_coords, lattice, mask, emb_table,
                 edge_w, edge_b, node_w, node_b):
    shared = _shared_inputs(edge_w, edge_b, node_w, node_b)
    cart = np.einsum('bnd,bde->bne', frac_coords, lattice).astype(np.float32)
    nsq = (cart * cart).sum(-1)                                   # (B, N)
    d2 = (nsq[:, :, None] + nsq[:, None, :]
          - 2.0 * np.einsum('bid,bjd->bij', cart, cart))
    d2 = np.maximum(d2, 0.0).astype(np.float32) + np.float32(1e-6)
    d = np.sqrt(d2)
    # rf64 [64, B*2048]: crystal c cols [c*2048,(c+1)*2048); fill f rows
    # 4f+2g+{0,1} = (d^2, d) of group g, i-rows [8f, 8f+8), row-major (i,j).
    rf = np.empty((64, B * FILLP), np.float32)
    for c in range(B):
        csl = slice(c * FILLP, (c + 1) * FILLP)
        for f in range(NFILL):
            for g in range(G):
                i0 = g * IPG + f * IPF
                rf[4 * f + 2 * g + 0, csl] = d2[c, i0:i0 + IPF].reshape(-1)
                rf[4 * f + 2 * g + 1, csl] = d[c, i0:i0 + IPF].reshape(-1)
    types = np.where(mask, atom_types, 0).astype(np.int64)        # (B, N)
    h0 = emb_table[types]                                         # (B, N, H)
    h0T = np.ascontiguousarray(
        h0.transpose(2, 0, 1).reshape(H, B * N)).astype(np.float32)
    maskF = np.broadcast_to(
        mask.astype(np.float32).reshape(1, B * N), (H, B * N)).copy()
    return [dict(rf64=rf, h0T=h0T, maskF=maskF, **shared)]


def kernel(**inputs):
    from concourse.bass_utils import run_bass_kernel_spmd

    atom_types = np.asarray(inputs["atom_types"])
    frac_coords = np.asarray(inputs["frac_coords"], np.float32)
    lattice = np.asarray(inputs["lattice"], np.float32)
    mask = np.asarray(inputs["mask"]).astype(bool)
    emb_table = np.asarray(inputs["emb_table"], np.float32)
    edge_w = np.asarray(inputs["edge_w"], np.float32)
    edge_b = np.asarray(inputs["edge_b"], np.float32)
    node_w = np.asarray(inputs["node_w"], np.float32)
    node_b = np.asarray(inputs["node_b"], np.float32)
    mu_w = np.asarray(inputs["mu_w"], np.float32)
    mu_b = np.asarray(inputs["mu_b"], np.float32)
    var_w = np.asarray(inputs["var_w"], np.float32)
    var_b = np.asarray(inputs["var_b"], np.float32)

    nc = _get_nc()
    in_maps = make_in_maps(atom_types, frac_coords, lattice, mask, emb_table,
                           edge_w, edge_b, node_w, node_b)
    res = run_bass_kernel_spmd(nc, in_maps, core_ids=[0])
    sum_h = np.ascontiguousarray(res.results[0]["sumh"].T)        # (B, H)
    n_valid = mask.sum(1).astype(np.float32)
    g = sum_h / (n_valid[:, None] + 1e-6)
    mu = (g @ mu_w + mu_b).astype(np.float32)
    log_var = (g @ var_w + var_b).astype(np.float32)
    return mu, log_var


# revision 16
# speedup vs baseline: 251.1307x; 1.3791x over previous
"""CrystalEncoder Trainium2 kernel (v4): all 8 crystals on ONE NeuronCore,
runtime-specialized to the ragged atom counts.

Why one core: in this axon environment each per-device NEFF dispatch carries
~1.2ms of launch overhead and the 8-device dispatch serializes them (~10ms
total — which is what the 9.25ms baseline number actually was). One dispatch
running all 8 crystals sequentially costs 1 launch + the compute.

Ragged specialization: lengths len_c (valid atoms) are in [N/2, N]. The
kernel is BUILT for the lengths observed in the inputs (cached per length
tuple; the build is pure emission, a few hundred ms):
  - j is trimmed to jp_c = ceil(len_c/32)*32 columns (host packs rf rows
    with jp pairs per i-row, so every on-device free dim scales by jp/N);
  - group-1 gate blocks whose 32 i-rows are entirely masked are skipped,
    and the node update / pooling only touch the first lp_c columns.
Invalid j inside jp contribute zero via h_j = 0 (padding embedding row);
invalid i inside lp are masked by maskF after the node update.

Per crystal (N=256, H=128, BINS=40, NL=2):
  1. rf64 slice: 16 fills x 4 rows (d^2/d x 2 i-groups), fill = 8 i-rows
     x jp pairs per group (f32r, host-computed).
  2. RBF exponents via K=64 matmuls (cE64 = 16 per-fill selector blocks),
     Exp bias -g*c_k^2 -> rbfT [128, 128*jp] bf16 (groups at partitions
     0/64, same free column = same (i_local, j) pair of each group).
  3. Per layer: gate matmuls (K=40 bf16, <=512-free, psum 8-i-row chunks);
     softplus = Exp then Ln(1+x) (one natural_log_exp table set); DVE 2x
     bf16 multiply by broadcast h_j + add-halves + reduce -> aggT; node
     update zT = node_w^T @ aggT + Silu + residual + mask.
  4. sum over atoms -> sumh column; one [H, 8] output DMA at the end.

Software pipelining: crystal c's layer-2 node update is deferred until
after crystal c+1's RBF stage, and layer-2's first two gate blocks are
produced before layer-1's node update, so ACT (the bottleneck engine)
never waits on DVE reduce tails. All element-wise work is on DVE (GpSimd
tensor ops are Q7 software at ~0.42 efficiency on real HW).

Sync discipline: this walrus build supports at most ONE semaphore wait per
instruction; _install_wait_splitter() splits multi-wait instructions with
same-engine NoOp carriers.
"""

import numpy as np
import ml_dtypes

B, N, H, LAT, NL, BINS = 8, 256, 128, 64, 2, 40
VMAX = 8.0
GAMMA = 1.0 / (VMAX / BINS) ** 2  # 25.0

G = 2                  # i-groups; bins at partition offsets 0 / 64
IPG = N // G           # 128 i-rows per group
NFILL = 16             # rf fills per crystal (8 i-rows per group each)
IPF = 8                # i-rows per fill per group
IPB = 32               # i-rows per gate block
IPC = 8                # i-rows per PSUM chunk
MMF = 512              # matmul free size (hard ISA limit)
RFSTRIDE = 2048        # rf columns reserved per crystal (>= IPF * jp)

_CACHE = {}


def _install_wait_splitter():
    """This walrus build supports at most ONE semaphore wait per ISA
    instruction. Split every multi-wait instruction by inserting same-engine
    NoOp carriers, each holding one of the waits, immediately before it."""
    import bass_rust
    import concourse.tile as tile
    from concourse import mybir

    if getattr(tile.TileContext, "_wait_split_installed", False):
        return
    orig = tile.TileContext._lower_ordered_insts
    counter = [0]

    def patched(self, ordered):
        for insts in ordered.values():
            newl = []
            for inst in insts:
                si = inst.sync_info
                ow = list(si.on_wait) if (si is not None and si.on_wait) else []
                if len(ow) > 1 and inst.engine != mybir.EngineType.Unassigned:
                    for w in ow[:-1]:
                        counter[0] += 1
                        nop = bass_rust.InstNoOp(
                            name=f"wsplit_{counter[0]}", ins=[], outs=[]
                        )
                        nop.engine = inst.engine
                        nop.sync_info = bass_rust.SyncInfo(
                            on_wait=[w], on_update=[]
                        )
                        newl.append(nop)
                    inst.sync_info = bass_rust.SyncInfo(
                        on_wait=[ow[-1]], on_update=list(si.on_update or [])
                    )
                newl.append(inst)
            insts[:] = newl
        return orig(self, ordered)

    tile.TileContext._lower_ordered_insts = patched

    def patched_dab(self, tick_clock, wait_clock):
        from concourse.vector_clock import ScopedClock

        probe = self.nc.sync.nop()
        wait_clock.add_sem_waits(
            probe.ins, ScopedClock({None: tick_clock.global_clock})
        )
        si = probe.ins.sync_info
        ow = list(si.on_wait) if (si is not None and si.on_wait) else []
        if len(ow) > 1:
            probe.ins.sync_info = bass_rust.SyncInfo(
                on_wait=[ow[0]], on_update=list(si.on_update or [])
            )
            for w in ow[1:]:
                n2 = self.nc.sync.nop()
                n2.ins.sync_info = bass_rust.SyncInfo(on_wait=[w], on_update=[])
        self.nc.sync.drain()
        self.nc.all_engine_barrier()
        popped = self.nc._tile_sem_poison_stack.pop()
        assert popped is self._sem_poison
        self.nc.clear_and_free_semaphores(list(self.sems.allocated().values()))
        self.nc.all_engine_barrier()

    tile.TileContext._drain_and_barrier = patched_dab
    tile.TileContext._wait_split_installed = True


def _crystal_geom(length):
    """Per-crystal specialization: (jp, g1blk, lp)."""
    jp = min(N, -(-int(length) // IPB) * IPB)       # j columns kept
    g1 = max(0, min(IPG, int(length) - IPG))        # valid group-1 i-rows
    g1blk = -(-g1 // IPB)                           # group-1 gate blocks
    lp = IPG + g1blk * IPB                          # i columns computed
    return jp, g1blk, lp


def _build_nc(lengths):
    import concourse.bass as bass
    import concourse.tile as tile
    from concourse import mybir

    _install_wait_splitter()

    F32 = mybir.dt.float32
    F32R = mybir.dt.float32r
    BF16 = mybir.dt.bfloat16
    AF = mybir.ActivationFunctionType
    X = mybir.AxisListType
    POOL = mybir.EngineType.Pool
    SP = mybir.EngineType.SP

    nc = bass.Bass("TRN2", target_bir_lowering=False, debug=False)

    def dep_nop(engine, aps):
        """Engine-local nop reading `aps`: pulls their producers' ticks into
        the engine's observed clock so later real instructions need at most
        one new semaphore wait."""
        nop = engine.nop(hint="dep").ins
        nop.ins = [engine.lower_ap(ap) for ap in aps]
        return nop

    NCR = len(lengths)
    d_rf = nc.dram_tensor("rf64", [64, NCR * RFSTRIDE], F32R,
                          kind="ExternalInput")
    d_cE = nc.dram_tensor("cE64", [64, NFILL * H], F32R, kind="ExternalInput")
    d_cbias = nc.dram_tensor("cbias", [H, 1], F32, kind="ExternalInput")
    d_ewR = nc.dram_tensor("ewR", [H, NL * H], BF16, kind="ExternalInput")
    d_ebT = nc.dram_tensor("ebT", [H, NL], F32, kind="ExternalInput")
    d_nwT = nc.dram_tensor("nwT", [H, NL * H], F32, kind="ExternalInput")
    d_nbT = nc.dram_tensor("nbT", [H, NL], F32, kind="ExternalInput")
    d_h0T = nc.dram_tensor("h0T", [H, NCR * N], F32, kind="ExternalInput")
    d_maskF = nc.dram_tensor("maskF", [H, NCR * N], F32, kind="ExternalInput")
    d_sumh = nc.dram_tensor("sumh", [H, NCR], F32, kind="ExternalOutput")

    with tile.TileContext(nc) as tc:
        with tc.tile_pool(name="consts", bufs=1) as consts, \
             tc.tile_pool(name="rfp", bufs=1) as rfp, \
             tc.tile_pool(name="lay", bufs=2) as lay, \
             tc.tile_pool(name="gxp", bufs=2) as gxp, \
             tc.tile_pool(name="gtp", bufs=2) as gtp, \
             tc.tile_pool(name="ppp", bufs=1) as ppp, \
             tc.tile_pool(name="tmp", bufs=1) as tmpp, \
             tc.tile_pool(name="ps", bufs=2, space="PSUM") as ps:
            kwp = dict(forced_dma_engine=POOL)
            kws = dict(forced_dma_engine=SP)
            t_cE = consts.tile_from(d_cE[:], **kws)
            t_cbias = consts.tile_from(d_cbias[:], **kwp)
            t_ewR = consts.tile_from(d_ewR[:], **kws)
            t_ebT = consts.tile_from(d_ebT[:], **kwp)
            t_nwT = consts.tile_from(d_nwT[:], **kws)
            t_nbT = consts.tile_from(d_nbT[:], **kwp)
            t_h = consts.tile_from(d_h0T[:], **kws)
            t_maskF = consts.tile_from(d_maskF[:], **kwp)

            rbfT = consts.tile([H, IPG * N], BF16)
            sumh = consts.tile([H, NCR], F32, tag="sumh")

            dep_nop(nc.tensor, [t_cE[:], t_ewR[:], t_nwT[:]])
            dep_nop(nc.scalar, [t_cbias[:], t_ebT[:], t_nbT[:]])
            dep_nop(nc.vector, [t_h[:], t_maskF[:]])

            def stage2(c, jp):
                """RBF table build for crystal c: rf DMA, K=64 exponent
                matmuls per fill, Exp -> rbfT[:, :128*jp]."""
                fw = IPF * jp                       # free width per fill
                rf = rfp.tile([64, RFSTRIDE], F32R, tag="rf")
                nc.gpsimd.dma_start(
                    out=rf[:, :fw],
                    in_=d_rf[:, c * RFSTRIDE:c * RFSTRIDE + fw])
                dep_nop(nc.tensor, [rf[:]])
                for f in range(NFILL):
                    e = ps.tile([H, IPC * N], F32, tag="ps")
                    for s in range(-(-fw // MMF)):
                        w = min(MMF, fw - s * MMF)
                        nc.tensor.matmul(
                            e[:, s * MMF:s * MMF + w],
                            t_cE[:, f * H:(f + 1) * H],
                            rf[:, s * MMF:s * MMF + w],
                            start=True, stop=True,
                        )
                    nc.scalar.activation(
                        rbfT[:, f * fw:(f + 1) * fw], e[:, :fw], AF.Exp,
                        bias=t_cbias[:],
                    )

            def gate_produce(l, b, jp):
                """Gate matmuls + Exp + Ln for one 32-i-row block."""
                g, bi = divmod(b, IPG // IPB)
                bw = IPB * jp                       # block free width
                cw = IPC * jp                       # psum chunk width
                lf = bi * bw
                gx = gxp.tile([H, IPB * N], BF16, tag="gx")
                for cch in range(IPB // IPC):
                    gp = ps.tile([H, IPC * N], F32, tag="ps")
                    for s in range(-(-cw // MMF)):
                        w = min(MMF, cw - s * MMF)
                        f0 = lf + cch * cw + s * MMF
                        nc.tensor.matmul(
                            gp[:, s * MMF:s * MMF + w],
                            t_ewR[64 * g:64 * g + BINS, l * H:(l + 1) * H],
                            rbfT[64 * g:64 * g + BINS, f0:f0 + w],
                            start=True, stop=True,
                        )
                    nc.scalar.activation(
                        gx[:, cch * cw:(cch + 1) * cw], gp[:, :cw], AF.Exp,
                        bias=t_ebT[:, l:l + 1],
                    )
                gt = gtp.tile([H, IPB * N], BF16, tag="gt")
                nc.scalar.activation(gt[:, :bw], gx[:, :bw], AF.Ln, bias=1.0)
                return gt

            def gate_consume(gt, b, jp, hmr, aggT, split=False):
                """DVE: pp = gt * h_j; add j-halves; reduce -> aggT cols."""
                g, bi = divmod(b, IPG // IPB)
                i0 = g * IPG + bi * IPB
                subs = (IPB // IPC) if split else 1
                rows = IPB // subs
                w = rows * jp
                pp = ppp.tile([H, IPB * N], BF16, tag="pp")
                tm = tmpp.tile([H, IPB * N // 2], BF16, tag="tm")
                for s in range(subs):
                    sl_ = slice(s * w, (s + 1) * w)
                    ppv = pp[:, sl_].rearrange("p (r c) -> p r c", c=jp)
                    nc.vector.tensor_mul(
                        ppv,
                        gt[:, sl_].rearrange("p (r c) -> p r c", c=jp),
                        hmr[:, None, :jp].broadcast_to([H, rows, jp]),
                    )
                    tmv = tm[:, s * w // 2:(s + 1) * w // 2].rearrange(
                        "p (r c) -> p r c", c=jp // 2)
                    nc.vector.tensor_add(
                        tmv, ppv[:, :, 0:jp // 2], ppv[:, :, jp // 2:jp])
                    nc.vector.reduce_sum(
                        out=aggT[:, i0 + s * rows:i0 + (s + 1) * rows],
                        in_=tmv, axis=X.X,
                    )

            def node_update(c, l, aggT, lp):
                """zT = node_w^T @ aggT; h += silu(zT + b); h *= mask.
                Only the first lp columns are computed columns."""
                hsl = slice(c * N, c * N + lp)
                dep_nop(nc.tensor, [aggT[:]])
                zp = ps.tile([H, IPC * N], F32, tag="ps")
                nc.tensor.matmul(
                    zp[:, :lp], t_nwT[:, l * H:(l + 1) * H], aggT[:, :lp],
                    start=True, stop=True,
                )
                sl = lay.tile([H, N], F32, tag="sil")
                nc.scalar.activation(
                    sl[:, :lp], zp[:, :lp], AF.Silu, bias=t_nbT[:, l:l + 1])
                h2 = lay.tile([H, N], F32, tag="h2")
                nc.vector.tensor_add(h2[:, :lp], t_h[:, hsl], sl[:, :lp])
                nc.vector.tensor_mul(t_h[:, hsl], h2[:, :lp],
                                     t_maskF[:, hsl])

            deferred = None  # (c, aggT2, lp) awaiting layer-2 node update

            def finish_crystal(dfr):
                c, aggT2, lp = dfr
                node_update(c, 1, aggT2, lp)
                nc.vector.reduce_sum(
                    out=sumh[:, c:c + 1], in_=t_h[:, c * N:(c + 1) * N],
                    axis=X.X,
                )

            for c in range(NCR):
                jp, g1blk, lp = _crystal_geom(lengths[c])
                nblk = IPG // IPB + g1blk           # gate blocks per layer
                stage2(c, jp)
                if deferred is not None:
                    finish_crystal(deferred)
                    deferred = None
                hsl = slice(c * N, (c + 1) * N)
                hmr1 = lay.tile([H, N], BF16, tag="hmr1")
                nc.vector.tensor_copy(hmr1[:], t_h[:, hsl])
                # layer 1
                aggT1 = lay.tile([H, N], F32, tag="agg1")
                for b in range(nblk):
                    gt = gate_produce(0, b, jp)
                    gate_consume(gt, b, jp, hmr1, aggT1, split=(b == nblk - 1))
                # layer 2: produce first two blocks before layer-1 node
                # update so ACT stays busy over the layer-1 reduce tail
                aggT2 = lay.tile([H, N], F32, tag="agg2")
                gt20 = gate_produce(1, 0, jp)
                gt21 = gate_produce(1, 1, jp)
                node_update(c, 0, aggT1, lp)
                hmr2 = lay.tile([H, N], BF16, tag="hmr2")
                nc.vector.tensor_copy(hmr2[:], t_h[:, hsl])
                gate_consume(gt20, 0, jp, hmr2, aggT2)
                gate_consume(gt21, 1, jp, hmr2, aggT2)
                for b in range(2, nblk):
                    gt = gate_produce(1, b, jp)
                    gate_consume(gt, b, jp, hmr2, aggT2, split=(b == nblk - 1))
                deferred = (c, aggT2, lp)

            finish_crystal(deferred)
            nc.gpsimd.dma_start(out=d_sumh[:], in_=sumh[:])

    return nc


def _get_nc(lengths):
    key = tuple(int(x) for x in lengths)
    if key not in _CACHE:
        _CACHE[key] = _build_nc(key)
    return _CACHE[key]


def _shared_inputs(edge_w, edge_b, node_w, node_b):
    centers = np.linspace(0.0, VMAX, BINS).astype(np.float64)
    # cE64: 16 per-fill selector blocks. Fill f uses rf rows 4f..4f+3:
    # row 4f+2g+0 = d^2 of group g, row 4f+2g+1 = d of group g.
    cE = np.zeros((64, NFILL * H), np.float32)
    for f in range(NFILL):
        for g in range(G):
            col0 = f * H + 64 * g
            cE[4 * f + 2 * g + 0, col0:col0 + BINS] = -GAMMA
            cE[4 * f + 2 * g + 1, col0:col0 + BINS] = 2.0 * GAMMA * centers
    cbias = np.zeros((H, 1), np.float32)
    ewR = np.zeros((H, NL * H), np.float32)
    for g in range(G):
        cbias[64 * g:64 * g + BINS, 0] = -GAMMA * centers * centers
        for l in range(NL):
            ewR[64 * g:64 * g + BINS, l * H:(l + 1) * H] = edge_w[l]
    ewR = ewR.astype(ml_dtypes.bfloat16)
    ebT = np.ascontiguousarray(edge_b.T).astype(np.float32)      # [H, NL]
    nwT = np.concatenate([node_w[l] for l in range(NL)], axis=1)
    nwT = np.ascontiguousarray(nwT).astype(np.float32)           # [H, NL*H]
    nbT = np.ascontiguousarray(node_b.T).astype(np.float32)      # [H, NL]
    return dict(cE64=cE, cbias=cbias, ewR=ewR, ebT=ebT, nwT=nwT, nbT=nbT)


def make_in_maps(atom_types, frac_coords, lattice, mask, emb_table,
                 edge_w, edge_b, node_w, node_b):
    shared = _shared_inputs(edge_w, edge_b, node_w, node_b)
    lengths = mask.sum(1).astype(int)
    cart = np.einsum('bnd,bde->bne', frac_coords, lattice).astype(np.float32)
    nsq = (cart * cart).sum(-1)                                   # (B, N)
    d2 = (nsq[:, :, None] + nsq[:, None, :]
          - 2.0 * np.einsum('bid,bjd->bij', cart, cart))
    d2 = np.maximum(d2, 0.0).astype(np.float32) + np.float32(1e-6)
    d = np.sqrt(d2)
    # rf64 [64, B*RFSTRIDE]: crystal c cols [c*RFSTRIDE, ...); fill f rows
    # 4f+2g+{0,1} = (d^2, d) of group g, i-rows [8f, 8f+8), j < jp_c,
    # row-major over (i, j).
    rf = np.zeros((64, B * RFSTRIDE), np.float32)
    for c in range(B):
        jp, _, _ = _crystal_geom(lengths[c])
        fw = IPF * jp
        for f in range(NFILL):
            for g in range(G):
                i0 = g * IPG + f * IPF
                csl = slice(c * RFSTRIDE, c * RFSTRIDE + fw)
                rf[4 * f + 2 * g + 0, csl] = \
                    d2[c, i0:i0 + IPF, :jp].reshape(-1)
                rf[4 * f + 2 * g + 1, csl] = \
                    d[c, i0:i0 + IPF, :jp].reshape(-1)
    types = np.where(mask, atom_types, 0).astype(np.int64)        # (B, N)
    h0 = emb_table[types]                                         # (B, N, H)
    h0T = np.ascontiguousarray(
        h0.transpose(2, 0, 1).reshape(H, B * N)).astype(np.float32)
    maskF = np.broadcast_to(
        mask.astype(np.float32).reshape(1, B * N), (H, B * N)).copy()
    return [dict(rf64=rf, h0T=h0T, maskF=maskF, **shared)]


def kernel(**inputs):
    from concourse.bass_utils import run_bass_kernel_spmd

    atom_types = np.asarray(inputs["atom_types"])
    frac_coords = np.asarray(inputs["frac_coords"], np.float32)
    lattice = np.asarray(inputs["lattice"], np.float32)
    mask = np.asarray(inputs["mask"]).astype(bool)
    emb_table = np.asarray(inputs["emb_table"], np.float32)
    edge_w = np.asarray(inputs["edge_w"], np.float32)
    edge_b = np.asarray(inputs["edge_b"], np.float32)
    node_w = np.asarray(inputs["node_w"], np.float32)
    node_b = np.asarray(inputs["node_b"], np.float32)
    mu_w = np.asarray(inputs["mu_w"], np.float32)
    mu_b = np.asarray(inputs["mu_b"], np.float32)
    var_w = np.asarray(inputs["var_w"], np.float32)
    var_b = np.asarray(inputs["var_b"], np.float32)

    lengths = mask.sum(1).astype(int)
    nc = _get_nc(lengths)
    in_maps = make_in_maps(atom_types, frac_coords, lattice, mask, emb_table,
                           edge_w, edge_b, node_w, node_b)
    res = run_bass_kernel_spmd(nc, in_maps, core_ids=[0])
    sum_h = np.ascontiguousarray(res.results[0]["sumh"].T)        # (B, H)
    n_valid = mask.sum(1).astype(np.float32)
    g = sum_h / (n_valid[:, None] + 1e-6)
    mu = (g @ mu_w + mu_b).astype(np.float32)
    log_var = (g @ var_w + var_b).astype(np.float32)
    return mu, log_var


# revision 21
# speedup vs baseline: 279.1349x; 1.1115x over previous
"""CrystalEncoder Trainium2 kernel (v4): all 8 crystals on ONE NeuronCore,
runtime-specialized to the ragged atom counts.

Why one core: in this axon environment each per-device NEFF dispatch carries
~1.2ms of launch overhead and the 8-device dispatch serializes them (~10ms
total — which is what the 9.25ms baseline number actually was). One dispatch
running all 8 crystals sequentially costs 1 launch + the compute.

Ragged specialization: lengths len_c (valid atoms) are in [N/2, N]. The
kernel is BUILT for the lengths observed in the inputs (cached per length
tuple; the build is pure emission, a few hundred ms):
  - j is trimmed to jp_c = ceil(len_c/32)*32 columns (host packs rf rows
    with jp pairs per i-row, so every on-device free dim scales by jp/N);
  - group-1 gate blocks whose 32 i-rows are entirely masked are skipped,
    and the node update / pooling only touch the first lp_c columns.
Invalid j inside jp contribute zero via h_j = 0 (padding embedding row);
invalid i inside lp are masked by maskF after the node update.

Per crystal (N=256, H=128, BINS=40, NL=2):
  1. rf64 slice: 16 fills x 4 rows (d^2/d x 2 i-groups), fill = 8 i-rows
     x jp pairs per group (f32r, host-computed).
  2. RBF exponents via K=64 matmuls (cE64 = 16 per-fill selector blocks),
     Exp bias -g*c_k^2 -> rbfT [128, 128*jp] bf16 (groups at partitions
     0/64, same free column = same (i_local, j) pair of each group).
  3. Per layer: gate matmuls (K=40 bf16, <=512-free, psum 8-i-row chunks);
     softplus = Exp then Ln(1+x) (one natural_log_exp table set); DVE 2x
     bf16 multiply by broadcast h_j + add-halves + reduce -> aggT; node
     update zT = node_w^T @ aggT + Silu + residual + mask.
  4. sum over atoms -> sumh column; one [H, 8] output DMA at the end.

Software pipelining: crystal c's layer-2 node update is deferred until
after crystal c+1's RBF stage, and layer-2's first two gate blocks are
produced before layer-1's node update, so ACT (the bottleneck engine)
never waits on DVE reduce tails. All element-wise work is on DVE (GpSimd
tensor ops are Q7 software at ~0.42 efficiency on real HW).

Sync discipline: this walrus build supports at most ONE semaphore wait per
instruction; _install_wait_splitter() splits multi-wait instructions with
same-engine NoOp carriers.
"""

import numpy as np
import ml_dtypes

B, N, H, LAT, NL, BINS = 8, 256, 128, 64, 2, 40
VMAX = 8.0
GAMMA = 1.0 / (VMAX / BINS) ** 2  # 25.0

G = 2                  # i-groups; bins at partition offsets 0 / 64
IPG = N // G           # 128 i-rows per group
NFILL = 16             # rf fills per crystal (8 i-rows per group each)
IPF = 8                # i-rows per fill per group
IPB = 32               # i-rows per gate block
IPC = 8                # i-rows per PSUM chunk
MMF = 512              # matmul free size (hard ISA limit)
RFSTRIDE = 2048        # rf columns reserved per crystal (>= IPF * jp)

_CACHE = {}


def _install_wait_splitter():
    """This walrus build supports at most ONE semaphore wait per ISA
    instruction. Split every multi-wait instruction by inserting same-engine
    NoOp carriers, each holding one of the waits, immediately before it."""
    import bass_rust
    import concourse.tile as tile
    from concourse import mybir

    if getattr(tile.TileContext, "_wait_split_installed", False):
        return
    orig = tile.TileContext._lower_ordered_insts
    counter = [0]

    def patched(self, ordered):
        for insts in ordered.values():
            newl = []
            for inst in insts:
                si = inst.sync_info
                ow = list(si.on_wait) if (si is not None and si.on_wait) else []
                if len(ow) > 1 and inst.engine != mybir.EngineType.Unassigned:
                    for w in ow[:-1]:
                        counter[0] += 1
                        nop = bass_rust.InstNoOp(
                            name=f"wsplit_{counter[0]}", ins=[], outs=[]
                        )
                        nop.engine = inst.engine
                        nop.sync_info = bass_rust.SyncInfo(
                            on_wait=[w], on_update=[]
                        )
                        newl.append(nop)
                    inst.sync_info = bass_rust.SyncInfo(
                        on_wait=[ow[-1]], on_update=list(si.on_update or [])
                    )
                newl.append(inst)
            insts[:] = newl
        return orig(self, ordered)

    tile.TileContext._lower_ordered_insts = patched

    def patched_dab(self, tick_clock, wait_clock):
        from concourse.vector_clock import ScopedClock

        probe = self.nc.sync.nop()
        wait_clock.add_sem_waits(
            probe.ins, ScopedClock({None: tick_clock.global_clock})
        )
        si = probe.ins.sync_info
        ow = list(si.on_wait) if (si is not None and si.on_wait) else []
        if len(ow) > 1:
            probe.ins.sync_info = bass_rust.SyncInfo(
                on_wait=[ow[0]], on_update=list(si.on_update or [])
            )
            for w in ow[1:]:
                n2 = self.nc.sync.nop()
                n2.ins.sync_info = bass_rust.SyncInfo(on_wait=[w], on_update=[])
        self.nc.sync.drain()
        self.nc.all_engine_barrier()
        popped = self.nc._tile_sem_poison_stack.pop()
        assert popped is self._sem_poison
        self.nc.clear_and_free_semaphores(list(self.sems.allocated().values()))
        self.nc.all_engine_barrier()

    tile.TileContext._drain_and_barrier = patched_dab
    tile.TileContext._wait_split_installed = True


def _crystal_geom(length):
    """Per-crystal specialization: (jp, blocks, lp).

    jp: j columns kept (even). blocks: [(g, i0_local, rows)] gate blocks —
    group 0 always 4x32 rows, group 1 in 32-row blocks plus an 8-granular
    remainder. lp = 128 + padded group-1 rows (i columns computed)."""
    length = int(length)
    jp = min(N, length + (length & 1))
    g1 = max(0, min(IPG, length - IPG))
    g1p = -(-g1 // IPC) * IPC
    blocks = [(0, i0, IPB) for i0 in range(0, IPG, IPB)]
    full, rem = divmod(g1p, IPB)
    for k in range(full):
        blocks.append((1, k * IPB, IPB))
    if rem:
        blocks.append((1, full * IPB, rem))
    lp = IPG + g1p
    return jp, blocks, lp


def _build_nc(lengths):
    import concourse.bass as bass
    import concourse.tile as tile
    from concourse import mybir

    _install_wait_splitter()

    F32 = mybir.dt.float32
    F32R = mybir.dt.float32r
    BF16 = mybir.dt.bfloat16
    AF = mybir.ActivationFunctionType
    X = mybir.AxisListType
    POOL = mybir.EngineType.Pool
    SP = mybir.EngineType.SP

    nc = bass.Bass("TRN2", target_bir_lowering=False, debug=False)

    def dep_nop(engine, aps):
        """Engine-local nop reading `aps`: pulls their producers' ticks into
        the engine's observed clock so later real instructions need at most
        one new semaphore wait."""
        nop = engine.nop(hint="dep").ins
        nop.ins = [engine.lower_ap(ap) for ap in aps]
        return nop

    NCR = len(lengths)
    d_rf = nc.dram_tensor("rf64", [64, NCR * RFSTRIDE], F32R,
                          kind="ExternalInput")
    d_cE = nc.dram_tensor("cE64", [64, NFILL * H], F32R, kind="ExternalInput")
    d_cbias = nc.dram_tensor("cbias", [H, 1], F32, kind="ExternalInput")
    d_ewR = nc.dram_tensor("ewR", [H, NL * H], BF16, kind="ExternalInput")
    d_ebT = nc.dram_tensor("ebT", [H, NL], F32, kind="ExternalInput")
    d_nwT = nc.dram_tensor("nwT", [H, NL * H], F32, kind="ExternalInput")
    d_nbT = nc.dram_tensor("nbT", [H, NL], F32, kind="ExternalInput")
    d_h0T = nc.dram_tensor("h0T", [H, NCR * N], F32, kind="ExternalInput")
    d_maskF = nc.dram_tensor("maskF", [H, NCR * N], F32, kind="ExternalInput")
    d_sumh = nc.dram_tensor("sumh", [H, NCR], F32, kind="ExternalOutput")

    with tile.TileContext(nc) as tc:
        with tc.tile_pool(name="consts", bufs=1) as consts, \
             tc.tile_pool(name="rfp", bufs=1) as rfp, \
             tc.tile_pool(name="lay", bufs=2) as lay, \
             tc.tile_pool(name="gxp", bufs=2) as gxp, \
             tc.tile_pool(name="gtp", bufs=2) as gtp, \
             tc.tile_pool(name="ppp", bufs=1) as ppp, \
             tc.tile_pool(name="tmp", bufs=1) as tmpp, \
             tc.tile_pool(name="ps", bufs=2, space="PSUM") as ps:
            kwp = dict(forced_dma_engine=POOL)
            kws = dict(forced_dma_engine=SP)
            t_cE = consts.tile_from(d_cE[:], **kws)
            t_cbias = consts.tile_from(d_cbias[:], **kwp)
            t_ewR = consts.tile_from(d_ewR[:], **kws)
            t_ebT = consts.tile_from(d_ebT[:], **kwp)
            t_nwT = consts.tile_from(d_nwT[:], **kws)
            t_nbT = consts.tile_from(d_nbT[:], **kwp)
            t_h = consts.tile_from(d_h0T[:], **kws)
            t_maskF = consts.tile_from(d_maskF[:], **kwp)

            rbfT = consts.tile([H, IPG * N], BF16)
            sumh = consts.tile([H, NCR], F32, tag="sumh")

            dep_nop(nc.tensor, [t_cE[:], t_ewR[:], t_nwT[:]])
            dep_nop(nc.scalar, [t_cbias[:], t_ebT[:], t_nbT[:]])
            dep_nop(nc.vector, [t_h[:], t_maskF[:]])

            def stage2(c, jp):
                """RBF table build for crystal c: rf DMA, K=64 exponent
                matmuls per fill, Exp -> rbfT[:, :128*jp]."""
                fw = IPF * jp                       # free width per fill
                rf = rfp.tile([64, RFSTRIDE], F32R, tag="rf")
                nc.gpsimd.dma_start(
                    out=rf[:, :fw],
                    in_=d_rf[:, c * RFSTRIDE:c * RFSTRIDE + fw])
                dep_nop(nc.tensor, [rf[:]])
                for f in range(NFILL):
                    e = ps.tile([H, IPC * N], F32, tag="ps")
                    for s in range(-(-fw // MMF)):
                        w = min(MMF, fw - s * MMF)
                        nc.tensor.matmul(
                            e[:, s * MMF:s * MMF + w],
                            t_cE[:, f * H:(f + 1) * H],
                            rf[:, s * MMF:s * MMF + w],
                            start=True, stop=True,
                        )
                    nc.scalar.activation(
                        rbfT[:, f * fw:(f + 1) * fw], e[:, :fw], AF.Exp,
                        bias=t_cbias[:],
                    )

            def gate_produce(l, blk, jp):
                """Gate matmuls + Exp + Ln for one (g, i0_local, rows)
                block."""
                g, i0l, rows = blk
                bw = rows * jp                      # block free width
                cw = IPC * jp                       # psum chunk width
                lf = i0l * jp
                gx = gxp.tile([H, IPB * N], BF16, tag="gx")
                for cch in range(rows // IPC):
                    gp = ps.tile([H, IPC * N], F32, tag="ps")
                    for s in range(-(-cw // MMF)):
                        w = min(MMF, cw - s * MMF)
                        f0 = lf + cch * cw + s * MMF
                        nc.tensor.matmul(
                            gp[:, s * MMF:s * MMF + w],
                            t_ewR[64 * g:64 * g + BINS, l * H:(l + 1) * H],
                            rbfT[64 * g:64 * g + BINS, f0:f0 + w],
                            start=True, stop=True,
                        )
                    nc.scalar.activation(
                        gx[:, cch * cw:(cch + 1) * cw], gp[:, :cw], AF.Exp,
                        bias=t_ebT[:, l:l + 1],
                    )
                gt = gtp.tile([H, IPB * N], BF16, tag="gt")
                nc.scalar.activation(gt[:, :bw], gx[:, :bw], AF.Ln, bias=1.0)
                return gt

            def gate_consume(gt, blk, jp, hmr, aggT, split=False):
                """DVE: pp = gt * h_j; add j-halves; reduce -> aggT cols."""
                g, i0l, rows = blk
                i0 = g * IPG + i0l
                subs = (rows // IPC) if split else 1
                rw = rows // subs
                w = rw * jp
                pp = ppp.tile([H, IPB * N], BF16, tag="pp")
                tm = tmpp.tile([H, IPB * N // 2], BF16, tag="tm")
                for s in range(subs):
                    sl_ = slice(s * w, (s + 1) * w)
                    ppv = pp[:, sl_].rearrange("p (r c) -> p r c", c=jp)
                    nc.vector.tensor_mul(
                        ppv,
                        gt[:, sl_].rearrange("p (r c) -> p r c", c=jp),
                        hmr[:, None, :jp].broadcast_to([H, rw, jp]),
                    )
                    tmv = tm[:, s * w // 2:(s + 1) * w // 2].rearrange(
                        "p (r c) -> p r c", c=jp // 2)
                    nc.vector.tensor_add(
                        tmv, ppv[:, :, 0:jp // 2], ppv[:, :, jp // 2:jp])
                    nc.vector.reduce_sum(
                        out=aggT[:, i0 + s * rw:i0 + (s + 1) * rw],
                        in_=tmv, axis=X.X,
                    )

            def node_update(c, l, aggT, lp):
                """zT = node_w^T @ aggT; h += silu(zT + b); h *= mask.
                silu(z) = z * exp(-ln(1 + exp(-z))) uses only the
                natural_log_exp table set — no ACT table switches.
                Only the first lp columns are computed columns."""
                hsl = slice(c * N, c * N + lp)
                dep_nop(nc.tensor, [aggT[:]])
                zp = ps.tile([H, IPC * N], F32, tag="ps")
                nc.tensor.matmul(
                    zp[:, :lp], t_nwT[:, l * H:(l + 1) * H], aggT[:, :lp],
                    start=True, stop=True,
                )
                # z = zp + node_b (fold bias into the first Exp's scale
                # trick is not possible: bias applies pre-function), so
                # add it on DVE first.
                zt = lay.tile([H, N], F32, tag="zt")
                nc.vector.tensor_scalar_add(
                    zt[:, :lp], zp[:, :lp], t_nbT[:, l:l + 1])
                # clamp so exp(-z) can't overflow; silu(z < -30) ~ 0 and
                # the final multiply uses the unclamped z
                ztc = lay.tile([H, N], F32, tag="ztc")
                nc.vector.tensor_scalar_max(ztc[:, :lp], zt[:, :lp], -30.0)
                u = lay.tile([H, N], F32, tag="sgu")
                nc.scalar.activation(u[:, :lp], ztc[:, :lp], AF.Exp,
                                     scale=-1.0)
                w = lay.tile([H, N], F32, tag="sgw")
                nc.scalar.activation(w[:, :lp], u[:, :lp], AF.Ln, bias=1.0)
                sg = lay.tile([H, N], F32, tag="sgs")
                nc.scalar.activation(sg[:, :lp], w[:, :lp], AF.Exp,
                                     scale=-1.0)
                sl = lay.tile([H, N], F32, tag="sil")
                nc.vector.tensor_mul(sl[:, :lp], zt[:, :lp], sg[:, :lp])
                h2 = lay.tile([H, N], F32, tag="h2")
                nc.vector.tensor_add(h2[:, :lp], t_h[:, hsl], sl[:, :lp])
                nc.vector.tensor_mul(t_h[:, hsl], h2[:, :lp],
                                     t_maskF[:, hsl])

            deferred = None  # (c, aggT2, lp) awaiting layer-2 node update

            def finish_crystal(dfr):
                c, aggT2, lp = dfr
                node_update(c, 1, aggT2, lp)
                nc.vector.reduce_sum(
                    out=sumh[:, c:c + 1], in_=t_h[:, c * N:(c + 1) * N],
                    axis=X.X,
                )

            # longest crystals first: the final (un-hideable) reduce tail
            # then belongs to the shortest crystal
            order = sorted(range(NCR), key=lambda c: -int(lengths[c]))
            for c in order:
                jp, blocks, lp = _crystal_geom(lengths[c])
                nblk = len(blocks)
                stage2(c, jp)
                if deferred is not None:
                    finish_crystal(deferred)
                    deferred = None
                hsl = slice(c * N, (c + 1) * N)
                hmr1 = lay.tile([H, N], BF16, tag="hmr1")
                nc.vector.tensor_copy(hmr1[:], t_h[:, hsl])
                # layer 1
                aggT1 = lay.tile([H, N], F32, tag="agg1")
                for b in range(nblk):
                    gt = gate_produce(0, blocks[b], jp)
                    gate_consume(gt, blocks[b], jp, hmr1, aggT1,
                                 split=(b == nblk - 1))
                # layer 2: produce first two blocks before layer-1 node
                # update so ACT stays busy over the layer-1 reduce tail
                aggT2 = lay.tile([H, N], F32, tag="agg2")
                gt20 = gate_produce(1, blocks[0], jp)
                gt21 = gate_produce(1, blocks[1], jp)
                node_update(c, 0, aggT1, lp)
                hmr2 = lay.tile([H, N], BF16, tag="hmr2")
                nc.vector.tensor_copy(hmr2[:], t_h[:, hsl])
                gate_consume(gt20, blocks[0], jp, hmr2, aggT2)
                gate_consume(gt21, blocks[1], jp, hmr2, aggT2)
                for b in range(2, nblk):
                    gt = gate_produce(1, blocks[b], jp)
                    gate_consume(gt, blocks[b], jp, hmr2, aggT2,
                                 split=(b == nblk - 1))
                deferred = (c, aggT2, lp)

            finish_crystal(deferred)
            nc.gpsimd.dma_start(out=d_sumh[:], in_=sumh[:])

    return nc


def _get_nc(lengths):
    key = tuple(int(x) for x in lengths)
    if key not in _CACHE:
        _CACHE[key] = _build_nc(key)
    return _CACHE[key]


def _shared_inputs(edge_w, edge_b, node_w, node_b):
    centers = np.linspace(0.0, VMAX, BINS).astype(np.float64)
    # cE64: 16 per-fill selector blocks. Fill f uses rf rows 4f..4f+3:
    # row 4f+2g+0 = d^2 of group g, row 4f+2g+1 = d of group g.
    cE = np.zeros((64, NFILL * H), np.float32)
    for f in range(NFILL):
        for g in range(G):
            col0 = f * H + 64 * g
            cE[4 * f + 2 * g + 0, col0:col0 + BINS] = -GAMMA
            cE[4 * f + 2 * g + 1, col0:col0 + BINS] = 2.0 * GAMMA * centers
    cbias = np.zeros((H, 1), np.float32)
    ewR = np.zeros((H, NL * H), np.float32)
    for g in range(G):
        cbias[64 * g:64 * g + BINS, 0] = -GAMMA * centers * centers
        for l in range(NL):
            ewR[64 * g:64 * g + BINS, l * H:(l + 1) * H] = edge_w[l]
    ewR = ewR.astype(ml_dtypes.bfloat16)
    ebT = np.ascontiguousarray(edge_b.T).astype(np.float32)      # [H, NL]
    nwT = np.concatenate([node_w[l] for l in range(NL)], axis=1)
    nwT = np.ascontiguousarray(nwT).astype(np.float32)           # [H, NL*H]
    nbT = np.ascontiguousarray(node_b.T).astype(np.float32)      # [H, NL]
    return dict(cE64=cE, cbias=cbias, ewR=ewR, ebT=ebT, nwT=nwT, nbT=nbT)


def make_in_maps(atom_types, frac_coords, lattice, mask, emb_table,
                 edge_w, edge_b, node_w, node_b):
    shared = _shared_inputs(edge_w, edge_b, node_w, node_b)
    lengths = mask.sum(1).astype(int)
    cart = np.einsum('bnd,bde->bne', frac_coords, lattice).astype(np.float32)
    nsq = (cart * cart).sum(-1)                                   # (B, N)
    d2 = (nsq[:, :, None] + nsq[:, None, :]
          - 2.0 * np.einsum('bid,bjd->bij', cart, cart))
    d2 = np.maximum(d2, 0.0).astype(np.float32) + np.float32(1e-6)
    d = np.sqrt(d2)
    # rf64 [64, B*RFSTRIDE]: crystal c cols [c*RFSTRIDE, ...); fill f rows
    # 4f+2g+{0,1} = (d^2, d) of group g, i-rows [8f, 8f+8), j < jp_c,
    # row-major over (i, j).
    rf = np.zeros((64, B * RFSTRIDE), np.float32)
    for c in range(B):
        jp, _, _ = _crystal_geom(lengths[c])
        fw = IPF * jp
        for f in range(NFILL):
            for g in range(G):
                i0 = g * IPG + f * IPF
                csl = slice(c * RFSTRIDE, c * RFSTRIDE + fw)
                rf[4 * f + 2 * g + 0, csl] = \
                    d2[c, i0:i0 + IPF, :jp].reshape(-1)
                rf[4 * f + 2 * g + 1, csl] = \
                    d[c, i0:i0 + IPF, :jp].reshape(-1)
    types = np.where(mask, atom_types, 0).astype(np.int64)        # (B, N)
    h0 = emb_table[types]                                         # (B, N, H)
    h0T = np.ascontiguousarray(
        h0.transpose(2, 0, 1).reshape(H, B * N)).astype(np.float32)
    maskF = np.broadcast_to(
        mask.astype(np.float32).reshape(1, B * N), (H, B * N)).copy()
    return [dict(rf64=rf, h0T=h0T, maskF=maskF, **shared)]


def kernel(**inputs):
    from concourse.bass_utils import run_bass_kernel_spmd

    atom_types = np.asarray(inputs["atom_types"])
    frac_coords = np.asarray(inputs["frac_coords"], np.float32)
    lattice = np.asarray(inputs["lattice"], np.float32)
    mask = np.asarray(inputs["mask"]).astype(bool)
    emb_table = np.asarray(inputs["emb_table"], np.float32)
    edge_w = np.asarray(inputs["edge_w"], np.float32)
    edge_b = np.asarray(inputs["edge_b"], np.float32)
    node_w = np.asarray(inputs["node_w"], np.float32)
    node_b = np.asarray(inputs["node_b"], np.float32)
    mu_w = np.asarray(inputs["mu_w"], np.float32)
    mu_b = np.asarray(inputs["mu_b"], np.float32)
    var_w = np.asarray(inputs["var_w"], np.float32)
    var_b = np.asarray(inputs["var_b"], np.float32)

    lengths = mask.sum(1).astype(int)
    nc = _get_nc(lengths)
    in_maps = make_in_maps(atom_types, frac_coords, lattice, mask, emb_table,
                           edge_w, edge_b, node_w, node_b)
    res = run_bass_kernel_spmd(nc, in_maps, core_ids=[0])
    sum_h = np.ascontiguousarray(res.results[0]["sumh"].T)        # (B, H)
    n_valid = mask.sum(1).astype(np.float32)
    g = sum_h / (n_valid[:, None] + 1e-6)
    mu = (g @ mu_w + mu_b).astype(np.float32)
    log_var = (g @ var_w + var_b).astype(np.float32)
    return mu, log_var
